# revision 37
# baseline (speedup 1.0000x reference)
"""Trainium2 Bass kernel: ViT-style dense transformer block (B=64,S=577,D=768,H=12).

Sharding: pure data-parallel over batch across 8 NeuronCores (8 batches/core,
no collectives).  Per core the kernel runs two phases:

Phase 1 (attention), software-pipelined across batches so the in-order PE
stream always has independent work while softmax exp runs on the scalar
engine:

  - LN1 is computed token-major ((x-mu)*rstd only -- the LN gain/bias are
    folded into Wq/Wk/Wv and their biases by one-time in-kernel matmuls),
    and the normalized bf16 activations are transposed to feature-major via
    the DMA XBAR transpose engine (no PE transposes, no PSUM->SBUF copies).
    XBAR transpose requires a fully-contiguous destination block, so h1T is
    laid out token-tile-major [128, NT, KK, 128] and matmul moving operands
    use 2D-free access patterns spanning token tiles.
  - Scores are computed transposed (scoresT[j,i] = k_j . q_i, K=64) with the
    two heads of a pair issued back-to-back on PE row groups 0-63/64-127 so
    they execute concurrently (2x PE throughput); both heads' score tiles
    land in one 4-bank PSUM tile consumed by a single paired exp activation.
  - Per (pair, j-tile) the QKV chains of the NEXT batch are interleaved
    into the emission stream so PE never waits on exp.
  - The softmax denominator comes from an all-ones column appended to V.
    The PV accumulator is copied to SBUF immediately (frees the PSUM slot),
    then reciprocal + partition-broadcast + multiply run off the critical
    path.
  - Biases (v/out-proj/fc2) are applied on the (slack) vector engine from
    partition-broadcast rows; x2 spills to DRAM scratch in bf16.

Phase 2 (MLP) processes 512-token chunks of the core's 8*577 tokens
(avoids per-batch 65-token tail waste): LN2 (gain/bias folded into W1/b1)
-> DMA-transpose -> fc1 + tanh-GELU (scalar engine, bias in-activation)
-> fc2 + residual.  fc1 of chunk c is emitted before fc2 of chunk c-1 so
PE never waits on the gelu tail; LN2 runs one chunk ahead.

All matmuls use bf16 operands with fp32 PSUM accumulation; the residual
stream is bf16 in SBUF/spill (error contribution ~0.1-0.4% vs the 2e-2
gate; measured rel err ~3.5e-3).
"""

import math
import numpy as np

import concourse.bass as bass
import concourse.mybir as mybir
import concourse.tile as tile

F32 = mybir.dt.float32
I32 = mybir.dt.int32
BF16 = mybir.dt.bfloat16
AF = mybir.ActivationFunctionType
OP = mybir.AluOpType
RSQRT_MAGIC = 0x5f3759df

B, S, D, H, DH = 64, 577, 768, 12, 64
FF = 4 * D
EPS = 1e-6
NCORES = 8
KK = D // 128           # 6 k-tiles over D
MFF = FF // 128         # 24 tiles over FF
NHP = H // 2            # 6 head pairs
SCALE = 1.0 / math.sqrt(DH)

P1_ONLY = False
GELU_IDENT = False
S_TILES = [(i * 128, min(128, S - i * 128)) for i in range((S + 127) // 128)]
NT = len(S_TILES)       # 5


def _bcast(ap):
    """[N] dram AP -> [128, N] partition-broadcast AP."""
    return bass.AP(tensor=ap.tensor, offset=ap.offset, ap=[[0, 128]] + list(ap.ap))


def _row(ap):
    """[N] dram AP -> [1, N] single-partition AP."""
    return bass.AP(tensor=ap.tensor, offset=ap.offset, ap=[[0, 1]] + list(ap.ap))


def _ln_stats_tile(nc, pool, x_sl, rows, mvb, i):
    """bn stats over the free dim (768) of x_sl[:rows] -> mvb[:, i, :]=(mu,var)."""
    stats = pool.tile([128, 3, 6], F32, tag="lnstats", name="lnstats")
    for sg in range(3):
        nc.vector.bn_stats(stats[:rows, sg, :], x_sl[:, 256 * sg:256 * (sg + 1)])
    nc.vector.bn_aggr(mvb[:rows, i, :], stats[:rows])


def _rsqrt_batch(nc, pool, mvb, n):
    """rstd[:, i] = 1/sqrt(var_i + EPS), magic-constant + 2 Newton iters on DVE."""
    veps = pool.tile([128, 8], F32, tag="lnveps", name="veps")
    nc.vector.tensor_scalar_add(veps[:, :n], mvb[:, 0:n, 1], EPS)
    hv = pool.tile([128, 8], F32, tag="lnhv", name="hv")
    nc.vector.tensor_scalar_mul(hv[:, :n], veps[:, :n], 0.5)
    y = pool.tile([128, 8], F32, tag="lnrstd", name="rstd_b")
    t = pool.tile([128, 8], F32, tag="lnnt", name="nt")
    nc.vector.tensor_scalar(t[:, :n].bitcast(I32), veps[:, :n].bitcast(I32),
                            1, None, op0=OP.arith_shift_right)
    nc.vector.tensor_scalar(y[:, :n].bitcast(I32), t[:, :n].bitcast(I32),
                            -1, RSQRT_MAGIC, op0=OP.mult, op1=OP.add)
    for _ in range(2):
        nc.vector.tensor_tensor(t[:, :n], y[:, :n], y[:, :n], OP.mult)
        nc.vector.tensor_tensor(t[:, :n], t[:, :n], hv[:, :n], OP.mult)
        nc.vector.tensor_scalar(t[:, :n], t[:, :n], -1.0, 1.5,
                                op0=OP.mult, op1=OP.add)
        nc.vector.tensor_tensor(y[:, :n], y[:, :n], t[:, :n], OP.mult)
    return y


def make_tensors(nc, bpc):
    def din(name, shape):
        return nc.dram_tensor(name, shape, F32, kind="ExternalInput").ap()

    t = {}
    t["x"] = din("x", [bpc, S, D]).flatten_outer_dims()
    for nm, shape in (("ln1_g", [D]), ("ln1_b", [D]),
                      ("Wq", [H, D, DH]), ("bq", [H, DH]),
                      ("Wk", [H, D, DH]), ("bk", [H, DH]),
                      ("Wv", [H, D, DH]), ("bv", [H, DH]),
                      ("Wo", [D, D]), ("bo", [D]),
                      ("ln2_g", [D]), ("ln2_b", [D]),
                      ("W1", [D, FF]), ("b1", [FF]),
                      ("W2", [FF, D]), ("b2", [D])):
        t[nm] = din(nm, shape)
    t["out"] = nc.dram_tensor("out", [bpc, S, D], F32,
                              kind="ExternalOutput").ap().flatten_outer_dims()
    return t


def build_block(nc: bass.Bass, bpc: int, T: dict, rep: int = 0):
    tok = bpc * S
    chunks = [(c0, min(512, tok - c0)) for c0 in range(0, tok, 512)]
    sfx = f"_r{rep}" if rep else ""

    x = T["x"]
    ln1_g, ln1_b = T["ln1_g"], T["ln1_b"]
    wq, bq, wk, bk, wv, bv = T["Wq"], T["bq"], T["Wk"], T["bk"], T["Wv"], T["bv"]
    wo, bo = T["Wo"], T["bo"]
    ln2_g, ln2_b = T["ln2_g"], T["ln2_b"]
    w1, b1, w2, b2 = T["W1"], T["b1"], T["W2"], T["b2"]
    out = T["out"]
    x2s = nc.dram_tensor(f"x2_scratch{sfx}", [tok, D], BF16, kind="Internal").ap()

    import contextlib
    with contextlib.ExitStack() as res:
        singles = res.enter_context(tc_pool(nc, f"singles{sfx}", 1))
        small = res.enter_context(tc_pool(nc, f"small{sfx}", 3))

        # per-partition LN vectors: [128, KK] feature-major
        ln_pps = {}
        for nm, src in (("ln1g", ln1_g), ("ln1b", ln1_b),
                        ("ln2g", ln2_g), ("ln2b", ln2_b)):
            t = singles.tile([128, KK], F32, name=f"{nm}_pp")
            nc.gpsimd.dma_start(t, src.rearrange("(kk p) -> p kk", p=128))
            ln_pps[nm] = t
        lnb1_bf = singles.tile([128, KK], BF16, name="lnb1_bf")
        nc.vector.tensor_copy(lnb1_bf, ln_pps["ln1b"])
        lnb2_bf = singles.tile([128, KK], BF16, name="lnb2_bf")
        nc.vector.tensor_copy(lnb2_bf, ln_pps["ln2b"])

        # q/k biases, feature-major per head pair
        bq_pp = singles.tile([128, NHP], F32, name="bq_pp")
        nc.gpsimd.dma_start(bq_pp, bq.rearrange("(hp two) e -> (two e) hp", two=2))
        bk_pp = singles.tile([128, NHP], F32, name="bk_pp")
        nc.gpsimd.dma_start(bk_pp, bk.rearrange("(hp two) e -> (two e) hp", two=2))
        b1_pp = singles.tile([128, MFF], F32, name="b1_pp")
        nc.gpsimd.dma_start(b1_pp, b1.rearrange("(m p) -> p m", p=128))

        # flat bias rows + partition-broadcast copies for the DVE bias adds
        bvf = singles.tile([1, D], F32, name="bvf")
        nc.gpsimd.dma_start(bvf, _row(bv.rearrange("h e -> (h e)")))
        bof = singles.tile([1, D], F32, name="bof")
        nc.gpsimd.dma_start(bof, _row(bo))
        b2f = singles.tile([1, D], F32, name="b2f")
        nc.gpsimd.dma_start(b2f, _row(b2))
        bofb = singles.tile([1, D], BF16, name="bofb")
        nc.vector.tensor_copy(bofb, bof)
        b2fb = singles.tile([1, D], BF16, name="b2fb")
        nc.vector.tensor_copy(b2fb, b2f)
        bo_bc = singles.tile([128, D], BF16, name="bo_bc")
        nc.gpsimd.partition_broadcast(bo_bc, bofb, channels=128)
        b2_bc = singles.tile([128, D], BF16, name="b2_bc")
        nc.gpsimd.partition_broadcast(b2_bc, b2fb, channels=128)

        # ================= phase 1: attention =================
        with contextlib.ExitStack() as p1:
            wpool = p1.enter_context(tc_pool(nc, f"wpool1{sfx}", 1))
            stage = p1.enter_context(tc_pool(nc, f"stage1{sfx}", 2))
            pmm = p1.enter_context(tc_pool(nc, f"pmm{sfx}", 2, space="PSUM"))
            psc = p1.enter_context(tc_pool(nc, f"psc{sfx}", 1, space="PSUM"))
            xpool = p1.enter_context(tc_pool(nc, f"xpool{sfx}", 3))
            xstg = p1.enter_context(tc_pool(nc, f"xstg{sfx}", 2))
            h1pool = p1.enter_context(tc_pool(nc, f"h1pool{sfx}", 2))
            qpool = p1.enter_context(tc_pool(nc, f"qpool{sfx}", 2))
            kpool = p1.enter_context(tc_pool(nc, f"kpool{sfx}", 2))
            vpool = p1.enter_context(tc_pool(nc, f"vpool{sfx}", 2))
            epool = p1.enter_context(tc_pool(nc, f"epool{sfx}", 9))
            atpool = p1.enter_context(tc_pool(nc, f"atpool{sfx}", 1))
            x2pool = p1.enter_context(tc_pool(nc, f"x2pool{sfx}", 1))
            hnpool = p1.enter_context(tc_pool(nc, f"hnpool{sfx}", 2))
            bcpool = p1.enter_context(tc_pool(nc, f"bcpool{sfx}", 2))

            wq_sb = wpool.tile([128, KK, NHP, 128], BF16, name="wq_sb")
            wk_sb = wpool.tile([128, KK, NHP, 128], BF16, name="wk_sb")
            wv_sb = wpool.tile([128, KK, D], BF16, name="wv_sb")
            wo_sb = wpool.tile([128, KK, D], BF16, name="wo_sb")
            r_v = singles.tile([1, D], F32, name="r_v")
            r_vb = singles.tile([1, D], BF16, name="r_vb")
            rv_bc = singles.tile([128, D], BF16, name="rv_bc")

            cur = {}   # batch-indexed live tiles
            et_tiles = {}

            def emit_ln(b):
                base = b * S
                x_sb = xpool.tile([128, NT, D], BF16, name="x_sb")
                h1T = h1pool.tile([128, NT, KK, 128], BF16, name="h1T")
                mvb = small.tile([128, NT, 2], F32, tag="mvb", name="mvb")
                nc.vector.memset(mvb, 1.0)
                for i, (t0, rows) in enumerate(S_TILES):
                    xs = xstg.tile([128, D], F32, tag="xs", name="xs")
                    nc.sync.dma_start(xs[:rows], x[base + t0: base + t0 + rows, :])
                    nc.vector.tensor_copy(x_sb[:rows, i, :], xs[:rows])
                    _ln_stats_tile(nc, small, x_sb[:rows, i, :], rows, mvb, i)
                rstd_b = _rsqrt_batch(nc, small, mvb, NT)
                for i, (t0, rows) in enumerate(S_TILES):
                    hn = hnpool.tile([128, D], BF16, tag="hn", name="hn")
                    if rows < 128:
                        nc.vector.memset(hn, 0.0)
                    nc.vector.tensor_scalar(hn[:rows], x_sb[:rows, i, :],
                                            mvb[:rows, i, 0:1],
                                            rstd_b[:rows, i:i + 1],
                                            op0=OP.subtract, op1=OP.mult)
                    teng = nc.sync if i % 2 == 0 else nc.scalar
                    teng.dma_start_transpose(h1T[:, i, :, :], hn)
                cur[("x", b)] = x_sb
                cur[("h1T", b)] = h1T
                cur[("q", b)] = qpool.tile([128, NHP, S], BF16, name="q_sb")
                cur[("k", b)] = kpool.tile([128, NHP, S], BF16, name="k_sb")
                cur[("v", b)] = vpool.tile([128, NT, H, DH + 1], BF16, name="v_aug")
                cur[("at", b)] = atpool.tile([128, KK, S], BF16, name="attnT")

            def emit_qk_half(b, hp, which, half):
                """half 0: kk 0..2, half 1: kk 3..5 + bias copy."""
                h1T = cur[("h1T", b)]
                wsb = wq_sb if which == 0 else wk_sb
                key = ("qkps", b, hp, which)
                if half == 0:
                    cur[key] = pmm.tile([128, D], F32, tag="mm", name="qk_ps")
                ps = cur[key]
                for kk in range(3 * half, 3 * half + 3):
                    nc.tensor.matmul(ps[:, 0:512], wsb[:, kk, hp, :],
                                     h1T[:, 0:4, kk, :],
                                     start=(kk == 0), stop=(kk == KK - 1))
                    nc.tensor.matmul(ps[:, 512:S], wsb[:, kk, hp, :],
                                     h1T[:, 4, kk, 0:S - 512],
                                     start=(kk == 0), stop=(kk == KK - 1))
                if half == 1:
                    dst = cur[("q", b)] if which == 0 else cur[("k", b)]
                    bpp = bq_pp if which == 0 else bk_pp
                    nc.vector.tensor_scalar_add(dst[:, hp, :], ps[:, 0:S],
                                                bpp[:, hp:hp + 1])
                    del cur[key]

            def emit_v(b, i):
                h1T = cur[("h1T", b)]
                t0, rows = S_TILES[i]
                ps = pmm.tile([128, D], F32, tag="mm", name="v_ps")
                for kk in range(KK):
                    for n0, nw in ((0, 512), (512, 256)):
                        nc.tensor.matmul(ps[:rows, n0:n0 + nw],
                                         h1T[:, i, kk, 0:rows],
                                         wv_sb[:, kk, n0:n0 + nw],
                                         start=(kk == 0), stop=(kk == KK - 1))
                v_aug = cur[("v", b)]
                nc.vector.tensor_tensor(
                    v_aug[:rows, i, :, 0:DH],
                    ps[:rows, :].rearrange("p (h e) -> p h e", h=H),
                    rv_bc[:rows, :].rearrange("p (h e) -> p h e", h=H), OP.add)
                nc.vector.memset(v_aug[:rows, i, :, DH:DH + 1], 1.0)

            def emit_scores(b, hp, j):
                """paired scores for heads (2hp, 2hp+1) on row groups 0/64 + one exp."""
                t0, rj = S_TILES[j]
                q_sb, k_sb = cur[("q", b)], cur[("k", b)]
                ps = psc.tile([128, 2, 1024], F32, tag="sc", name="sc_ps")
                for n0, nw in ((0, 512), (512, S - 512)):
                    for o in range(2):
                        off = 64 * o
                        nc.tensor.matmul(ps[:rj, o, n0:n0 + nw],
                                         k_sb[off:off + DH, hp, t0:t0 + rj],
                                         q_sb[off:off + DH, hp, n0:n0 + nw],
                                         start=True, stop=True)
                et = epool.tile([128, 2, 640], BF16, tag="expT", name="expT")
                nc.scalar.activation(et[:rj, :, 0:S], ps[:rj, :, 0:S],
                                     AF.Exp, bias=0.0, scale=SCALE)
                et_tiles[(b, hp, j)] = et

            def emit_pv(b, hp):
                v_aug = cur[("v", b)]
                attnT = cur[("at", b)]
                for o in range(2):
                    h = 2 * hp + o
                    aps = pmm.tile([128, D], F32, tag="mm", name="attn_ps")
                    for n0, nw in ((0, 512), (512, S - 512)):
                        for j, (t0, rj) in enumerate(S_TILES):
                            nc.tensor.matmul(aps[0:DH + 1, n0:n0 + nw],
                                             v_aug[:rj, j, h, :],
                                             et_tiles[(b, hp, j)][:rj, o, n0:n0 + nw],
                                             start=(j == 0), stop=(j == NT - 1))
                    avs = bcpool.tile([DH + 1, S], F32, tag="avs", name="avs")
                    nc.vector.tensor_copy(avs, aps[0:DH + 1, 0:S])
                    rec = small.tile([1, S], F32, tag="rec", name="rec")
                    nc.vector.reciprocal(rec, avs[DH:DH + 1])
                    rec_bc = bcpool.tile([DH, S], F32, tag="recbc", name="rec_bc")
                    nc.gpsimd.partition_broadcast(rec_bc, rec, channels=DH)
                    nc.vector.tensor_tensor(attnT[64 * o:64 * o + DH, hp, :],
                                            avs[0:DH], rec_bc, OP.mult)
                for j in range(NT):
                    del et_tiles[(b, hp, j)]

            def emit_outproj(b):
                base = b * S
                attnT = cur.pop(("at", b))
                x_sb = cur.pop(("x", b))
                x2t = x2pool.tile([128, NT, D], BF16, name="x2t")
                for i, (t0, rows) in enumerate(S_TILES):
                    ops = pmm.tile([128, D], F32, tag="mm", name="op_ps")
                    for kk in range(KK):
                        for n0, nw in ((0, 512), (512, 256)):
                            nc.tensor.matmul(ops[:rows, n0:n0 + nw],
                                             attnT[:, kk, t0:t0 + rows],
                                             wo_sb[:, kk, n0:n0 + nw],
                                             start=(kk == 0), stop=(kk == KK - 1))
                    nc.vector.tensor_tensor(x2t[:rows, i, :], ops[:rows, :],
                                            x_sb[:rows, i, :], OP.add)
                    nc.vector.tensor_tensor(x2t[:rows, i, :], x2t[:rows, i, :],
                                            bo_bc[:rows, :], OP.add)
                    nc.sync.dma_start(x2s[base + t0: base + t0 + rows, :],
                                      x2t[:rows, i, :])
                cur.pop(("h1T", b))
                cur.pop(("q", b))
                cur.pop(("k", b))
                cur.pop(("v", b))

            # ---- startup: first LN, then weight staging (wq/wk first, split
            # across both HWDGE queues, so batch 0's q/k chains start early;
            # wv/wo stage while those run) ----
            emit_ln(0)
            for dst, wsrc, eng in ((wq_sb, wq, nc.sync), (wk_sb, wk, nc.scalar)):
                for hp in range(NHP):
                    st = stage.tile([128, KK, 128], F32, tag="stage", name="wqk_st")
                    for two in range(2):
                        eng.dma_start(
                            st[:, :, 64 * two:64 * two + 64],
                            wsrc[2 * hp + two].rearrange("(kk p) e -> p kk e", p=128))
                    for kk in range(KK):
                        nc.vector.tensor_scalar(dst[:, kk, hp, :], st[:, kk, :],
                                                ln_pps["ln1g"][:, kk:kk + 1], None,
                                                op0=OP.mult)

            def emit_qk_fold(hp):
                """bq/bk += Wq'^T ln1_b (per head pair, [128,1] psum chains)."""
                for bpp, wsb in ((bq_pp, wq_sb), (bk_pp, wk_sb)):
                    ps = pmm.tile([128, D], F32, tag="mm", name="fold_ps")
                    for kk in range(KK):
                        nc.tensor.matmul(ps[:, 0:1], wsb[:, kk, hp, :],
                                         lnb1_bf[:, kk:kk + 1],
                                         start=(kk == 0), stop=(kk == KK - 1))
                    nc.vector.tensor_tensor(bpp[:, hp:hp + 1], bpp[:, hp:hp + 1],
                                            ps[:, 0:1], OP.add)

            for h in range(H):
                st = stage.tile([128, KK, DH], F32, tag="stage", name="wv_st")
                eng = nc.sync if h % 2 == 0 else nc.scalar
                eng.dma_start(st, wv[h].rearrange("(kk p) e -> p kk e", p=128))
                for kk in range(KK):
                    nc.vector.tensor_scalar(wv_sb[:, kk, DH * h:DH * h + DH],
                                            st[:, kk, :],
                                            ln_pps["ln1g"][:, kk:kk + 1], None,
                                            op0=OP.mult)
            for kk in range(KK):
                st = stage.tile([128, D], F32, tag="stage", name="wo_st")
                eng = nc.sync if kk % 2 == 0 else nc.scalar
                eng.dma_start(st, wo[128 * kk:128 * (kk + 1), :])
                nc.vector.tensor_copy(wo_sb[:, kk, :], st)

            def emit_rv_fold():
                """rv_bc = broadcast(bv_flat + Wv'^T ln1_b)."""
                ps = pmm.tile([128, D], F32, tag="mm", name="rv_ps")
                for kk in range(KK):
                    for n0, nw in ((0, 512), (512, 256)):
                        nc.tensor.matmul(ps[0:1, n0:n0 + nw], lnb1_bf[:, kk:kk + 1],
                                         wv_sb[:, kk, n0:n0 + nw],
                                         start=(kk == 0), stop=(kk == KK - 1))
                nc.vector.tensor_tensor(r_v, ps[0:1, :], bvf, OP.add)
                nc.vector.tensor_copy(r_vb, r_v)
                nc.gpsimd.partition_broadcast(rv_bc, r_vb, channels=128)

            # ---- software-pipelined batch loop ----
            for b in range(bpc + 1):
                for hp in range(NHP):
                    if hp == 3 and b + 1 < bpc:
                        emit_ln(b + 1)
                    if b == 0:
                        emit_qk_fold(hp)
                    if b > 0:
                        emit_scores(b - 1, hp, 0)
                    if b < bpc:
                        emit_qk_half(b, hp, 0, 0)
                    if b > 0:
                        emit_scores(b - 1, hp, 1)
                    if b < bpc:
                        emit_qk_half(b, hp, 0, 1)
                    if b > 0:
                        emit_scores(b - 1, hp, 2)
                    if b < bpc:
                        emit_qk_half(b, hp, 1, 0)
                    if b > 0:
                        emit_scores(b - 1, hp, 3)
                    if b < bpc:
                        emit_qk_half(b, hp, 1, 1)
                    if b > 0:
                        emit_scores(b - 1, hp, 4)
                    if 0 < b < bpc and hp < NT:
                        emit_v(b, hp)
                    if b > 0 and hp > 0:
                        emit_pv(b - 1, hp - 1)
                if b == 0:
                    emit_rv_fold()
                    for i in range(NT):
                        emit_v(0, i)
                if b > 0:
                    emit_pv(b - 1, NHP - 1)
                    emit_outproj(b - 1)

        if P1_ONLY:
            return nc
        # ================= phase 2: MLP (512-token chunks) =================
        with contextlib.ExitStack() as p2:
            p1ps = p2.enter_context(tc_pool(nc, f"p1ps{sfx}", 3, space="PSUM"))
            p2ps = p2.enter_context(tc_pool(nc, f"p2ps{sfx}", 2, space="PSUM"))
            stage = p2.enter_context(tc_pool(nc, f"stage2{sfx}", 2))
            w1pool = p2.enter_context(tc_pool(nc, f"w1pool{sfx}", 1))
            w2pool = p2.enter_context(tc_pool(nc, f"w2pool{sfx}", 1))
            x2cpool = p2.enter_context(tc_pool(nc, f"x2cpool{sfx}", 3))
            h2pool = p2.enter_context(tc_pool(nc, f"h2pool{sfx}", 2))
            hnpool2 = p2.enter_context(tc_pool(nc, f"hnpool2{sfx}", 3))
            mpool = p2.enter_context(tc_pool(nc, f"mpool{sfx}", 2))
            opool = p2.enter_context(tc_pool(nc, f"opool{sfx}", 2))

            # stage W1 (ln2_g folded) / W2
            w1_sb = w1pool.tile([128, KK, MFF, 128], BF16, name="w1_sb")
            for kk in range(KK):
                for half in range(2):
                    st = stage.tile([128, FF // 2], F32, tag="stage", name="w1_st")
                    eng = nc.sync if half == 0 else nc.scalar
                    eng.dma_start(
                        st, w1[128 * kk:128 * (kk + 1),
                               (FF // 2) * half:(FF // 2) * (half + 1)])
                    nc.vector.tensor_scalar(
                        w1_sb[:, kk, 12 * half:12 * (half + 1), :]
                        .rearrange("p m e -> p (m e)"), st,
                        ln_pps["ln2g"][:, kk:kk + 1], None, op0=OP.mult)
            w2_sb = w2pool.tile([128, MFF, D], BF16, name="w2_sb")
            for m in range(MFF):
                st = stage.tile([128, D], F32, tag="stage", name="w2_st")
                eng = nc.sync if m % 2 == 0 else nc.scalar
                eng.dma_start(st, w2[128 * m:128 * (m + 1), :])
                nc.vector.tensor_copy(w2_sb[:, m, :], st)
            # fold ln2_b: b1 += W1'^T ln2_b
            for m in range(MFF):
                ps = p2ps.tile([128, D], F32, tag="mm2", name="b1f_ps")
                for kk in range(KK):
                    nc.tensor.matmul(ps[:, 0:1], w1_sb[:, kk, m, :],
                                     lnb2_bf[:, kk:kk + 1],
                                     start=(kk == 0), stop=(kk == KK - 1))
                nc.vector.tensor_tensor(b1_pp[:, m:m + 1], b1_pp[:, m:m + 1],
                                        ps[:, 0:1], OP.add)

            cur2 = {}

            def emit_ln2(c):
                c0, cw = chunks[c]
                ctiles = [(i0, min(128, cw - i0)) for i0 in range(0, cw, 128)]
                x2c = x2cpool.tile([128, 4, D], BF16, name="x2c")
                h2T = h2pool.tile([128, 4, KK, 128], BF16, name="h2T")
                mvb = small.tile([128, 4, 2], F32, tag="mvb2", name="mvb2")
                nc.vector.memset(mvb, 1.0)
                for i, (i0, rows) in enumerate(ctiles):
                    nc.sync.dma_start(x2c[:rows, i, :],
                                      x2s[c0 + i0: c0 + i0 + rows, :])
                    _ln_stats_tile(nc, small, x2c[:rows, i, :], rows, mvb, i)
                rstd_b = _rsqrt_batch(nc, small, mvb, len(ctiles))
                for i, (i0, rows) in enumerate(ctiles):
                    hn = hnpool2.tile([128, D], BF16, tag="hn2", name="hn2")
                    if rows < 128:
                        nc.vector.memset(hn, 0.0)
                    nc.vector.tensor_scalar(hn[:rows], x2c[:rows, i, :],
                                            mvb[:rows, i, 0:1],
                                            rstd_b[:rows, i:i + 1],
                                            op0=OP.subtract, op1=OP.mult)
                    teng = nc.sync if i % 2 == 0 else nc.scalar
                    teng.dma_start_transpose(h2T[:, i, :, :], hn)
                cur2[("x2c", c)] = x2c
                cur2[("h2T", c)] = h2T

            def emit_fc1(c):
                c0, cw = chunks[c]
                h2T = cur2[("h2T", c)]
                m_sb = mpool.tile([128, MFF, 512], BF16, name="m_sb")
                nfull, rem = divmod(cw, 128)
                for m in range(MFF):
                    fps = p1ps.tile([128, 512], F32, tag="fc1", name="fc1_ps")
                    for kk in range(KK):
                        if nfull:
                            nc.tensor.matmul(fps[:, 0:nfull * 128],
                                             w1_sb[:, kk, m, :],
                                             h2T[:, 0:nfull, kk, :],
                                             start=(kk == 0),
                                             stop=(kk == KK - 1 and not rem))
                        if rem:
                            nc.tensor.matmul(fps[:, nfull * 128:cw],
                                             w1_sb[:, kk, m, :],
                                             h2T[:, nfull, kk, 0:rem],
                                             start=(kk == 0 and not nfull),
                                             stop=(kk == KK - 1))
                    nc.scalar.activation(m_sb[:, m, 0:cw], fps[:, 0:cw],
                                         AF.Identity if GELU_IDENT
                                         else AF.Gelu_apprx_tanh,
                                         bias=b1_pp[:, m:m + 1], scale=1.0)
                cur2[("m", c)] = m_sb

            def emit_fc2(c):
                c0, cw = chunks[c]
                ctiles = [(i0, min(128, cw - i0)) for i0 in range(0, cw, 128)]
                m_sb = cur2.pop(("m", c))
                x2c = cur2.pop(("x2c", c))
                cur2.pop(("h2T", c))
                for i, (i0, rows) in enumerate(ctiles):
                    gps = p2ps.tile([128, D], F32, tag="mm2", name="fc2_ps")
                    for m in range(MFF):
                        for n0, nw in ((0, 512), (512, 256)):
                            nc.tensor.matmul(gps[:rows, n0:n0 + nw],
                                             m_sb[:, m, i0:i0 + rows],
                                             w2_sb[:, m, n0:n0 + nw],
                                             start=(m == 0), stop=(m == MFF - 1))
                    ot = opool.tile([128, D], F32, tag="ot", name="ot")
                    nc.vector.tensor_tensor(ot[:rows], gps[:rows],
                                            x2c[:rows, i, :], OP.add)
                    nc.vector.tensor_tensor(ot[:rows], ot[:rows],
                                            b2_bc[:rows], OP.add)
                    nc.sync.dma_start(out[c0 + i0: c0 + i0 + rows, :], ot[:rows, :])

            emit_ln2(0)
            for c in range(len(chunks) + 1):
                if c + 1 < len(chunks):
                    emit_ln2(c + 1)
                if c < len(chunks):
                    emit_fc1(c)
                if c > 0:
                    emit_fc2(c - 1)
    return nc


def tc_pool(nc, name, bufs, space="SBUF"):
    return nc.tc.tile_pool(name=name, bufs=bufs, space=space)


def build_nc(bpc=B // NCORES, reps=1):
    from concourse import bacc
    nc = bacc.Bacc("TRN2", target_bir_lowering=False, debug=False)
    with tile.TileContext(nc) as tc:
        nc.tc = tc
        T = make_tensors(nc, bpc)
        for rep in range(reps):
            build_block(nc, bpc, T, rep)
    nc.compile()
    return nc


_NC_CACHE = {}


def get_nc(bpc=B // NCORES, reps=1):
    key = (bpc, reps)
    if key not in _NC_CACHE:
        _NC_CACHE[key] = build_nc(bpc, reps)
    return _NC_CACHE[key]


def run(inputs, reps=1, **spmd_kwargs):
    from concourse.bass_utils import run_bass_kernel_spmd

    inputs = {k: np.ascontiguousarray(np.asarray(v, dtype=np.float32))
              for k, v in inputs.items()}
    x_full = inputs["x"]
    bpc = B // NCORES
    nc = get_nc(bpc, reps)
    weights = {k: v for k, v in inputs.items() if k != "x"}
    in_maps = [dict(weights, x=np.ascontiguousarray(x_full[c * bpc:(c + 1) * bpc]))
               for c in range(NCORES)]
    res = run_bass_kernel_spmd(nc, in_maps, core_ids=list(range(NCORES)),
                               **spmd_kwargs)
    out = np.concatenate([r["out"] for r in res.results], axis=0)
    return out, res


def kernel(**inputs):
    return run(inputs)[0]
